# revision 1
# baseline (speedup 1.0000x reference)
"""Multi-head self-attention (B=4, S=2048, D=1024, H=16) on 8 TRN2 NeuronCores.

Sharding: head-pair tensor parallel + token-sharded I/O. Core c owns heads
{2c, 2c+1} for ALL batches; weights shard 8-way with zero duplication. The
input x is shipped 1/8 per core (1024 tokens, transposed, fp16) and
AllGathered on-chip; each core computes QKV, attention and its 2-head partial
projection for all 4 batches; a per-batch ReduceScatter sums the partials and
leaves each core with 256 final rows per batch (fp16). Host adds b_proj.

All host<->device I/O is fp16 (~5 MB/core vs 24.3 MB for the v0 kernel) —
per-run staging of kernel I/O dominates measured time at ~14 GB/s aggregate.

Per-core dataflow per batch b (matmuls fp16 operands, fp32 PSUM):
  stage 1: yt_q/yt_k = [Q^T;K^T] [128f, 2048t], vp = V+[bias|ones] [2048t,130]
           (emitted in 4 chunks, dripped between attention query sweeps)
  stage 2: S^T[k,q] for the head pair (d=64 contraction), exp on ACT -> fp16
  stage 3: C~^T = [V_h|1]^T P^T (psum row 64 = softmax denom);
           recip -> gpsimd partition-broadcast -> DVE normalize -> ct fp16
  stage 4: out_partial = ct^T @ wp -> fp16 -> DRAM, dripped between S/exp
           steps; ReduceScatter(batch) once its last tile lands
"""
import numpy as np

import concourse.bacc as bacc
import concourse.tile as tile
from concourse import bass_isa, mybir
from concourse import bass_utils

P = 128
B, S, D = 4, 2048, 1024
H_TOT, HD = 16, 64
SCALE = HD ** -0.5
SH_T = 1024        # tokens per shard (B*S/8)
DCH = D // P       # 8 contraction chunks
NTT = S // P       # 16 token tiles per batch
f32 = mybir.dt.float32
f16 = mybir.dt.float16
AF = mybir.ActivationFunctionType
RG8 = [[0, 1, 2, 3, 4, 5, 6, 7]]

_CACHED_NC = None


# packed single-input blob layout (f16 element offsets)
OFF_XS = 0
OFF_WQK = OFF_XS + D * SH_T
OFF_WV = OFF_WQK + D * 2 * P
OFF_WP = OFF_WV + D * P
OFF_BQK = OFF_WP + P * D
OFF_VB = OFF_BQK + P * 2
NBLOB = OFF_VB + P * 130


def build_nc(reps=1):
    nc = bacc.Bacc(trn_type="TRN2", target_bir_lowering=False, debug=False,
                   num_devices=8, enable_partition_id=False)
    blob = nc.dram_tensor("blob", [1, NBLOB], f16, kind="ExternalInput").ap()
    xs = blob[0:1, OFF_XS:OFF_WQK]
    wqk = blob[0:1, OFF_WQK:OFF_WV]
    wv = blob[0:1, OFF_WV:OFF_WP]
    wp = blob[0:1, OFF_WP:OFF_BQK]
    bqk = blob[0:1, OFF_BQK:OFF_VB]
    vb = blob[0:1, OFF_VB:NBLOB]
    out = nc.dram_tensor("out", [B * 256, D], f16, kind="ExternalOutput").ap()

    ag_in = nc.dram_tensor("ag_in", [D, SH_T], f16, kind="Internal").ap()
    ag_out = nc.dram_tensor("ag_out", [8 * D, SH_T], f16, kind="Internal",
                            addr_space="Shared").ap()
    rs_in = [nc.dram_tensor(f"rs_in{b}", [S, D], f16, kind="Internal").ap()
             for b in range(B)]
    rs_out = [nc.dram_tensor(f"rs_out{b}", [256, D], f16,
                             kind="Internal").ap() for b in range(B)]

    with tile.TileContext(nc) as tc:
        with tc.tile_pool(name="persist", bufs=1) as pp:
            # double-buffered per-batch persistent tensors (b%2)
            ytq = [pp.tile([P, S], f16, name=f"ytq{i}") for i in range(2)]
            ytk = [pp.tile([P, S], f16, name=f"ytk{i}") for i in range(2)]
            vp = [pp.tile([P, NTT, 130], f16, name=f"vp{i}") for i in range(2)]
            ct = [pp.tile([P, S], f16, name=f"ct{i}") for i in range(2)]
            wqk_t = pp.tile([P, DCH, 2 * P], f16, name="wqk_t")
            wv_t = pp.tile([P, DCH, P], f16, name="wv_t")
            wp_t = pp.tile([P, D], f16, name="wp_t")
            bqk16 = pp.tile([P, 2], f16, name="bqk16")
            vb16 = pp.tile([P, 130], f16, name="vb16")
            bqk_t = pp.tile([P, 2], f32, name="bqk_t")
            vb_t = pp.tile([P, 130], f32, name="vb_t")

            # benchmarking support: reps>1 re-executes the whole body
            # (AllGather + 4 batches + ReduceScatters) serially
            for _rep in range(reps):
                # stage 0: weights to SBUF; x shard -> internal dram -> AllGather
                nc.scalar.dma_start(wqk_t[:],
                                    wqk.rearrange("o (c p f) -> p (o c) f",
                                                  p=P, f=2 * P))
                nc.scalar.dma_start(wv_t[:],
                                    wv.rearrange("o (c p f) -> p (o c) f",
                                                 p=P, f=P))
                nc.scalar.dma_start(wp_t[:],
                                    wp.rearrange("o (p f) -> (o p) f", p=P))
                nc.scalar.dma_start(bqk16[:],
                                    bqk.rearrange("o (p a) -> (o p) a", p=P))
                nc.scalar.dma_start(vb16[:],
                                    vb.rearrange("o (p a) -> (o p) a", p=P))
                nc.vector.tensor_copy(bqk_t[:], bqk16[:])
                nc.vector.tensor_copy(vb_t[:], vb16[:])
                with tc.tile_pool(name="agb", bufs=1) as agb:
                    agt = agb.tile([P, DCH, SH_T], f16, name="agt")
                    nc.sync.dma_start(agt[:],
                                      xs.rearrange("o (c p t) -> p (o c) t",
                                                   p=P, t=SH_T))
                    nc.sync.dma_start(ag_in.rearrange("(c p) t -> p c t", p=P),
                                      agt[:])
                nc.gpsimd.collective_compute(
                    "AllGather", mybir.AluOpType.bypass, replica_groups=RG8,
                    ins=[ag_in[:]], outs=[ag_out[:]])
                # the softmax-denominator ones columns of vp (written once)
                for i in range(2):
                    nc.vector.memset(vp[i][:, :, HD:HD + 1], 1.0)
                    nc.vector.memset(vp[i][:, :, 129:130], 1.0)

                with (
                    tc.tile_pool(name="s1x", bufs=2) as s1x,
                    tc.tile_pool(name="s1ps", bufs=1, space="PSUM") as s1ps,
                    tc.tile_pool(name="att", bufs=1) as att,
                    tc.tile_pool(name="s4o", bufs=2) as s4o,
                    tc.tile_pool(name="spt", bufs=2, space="PSUM") as sptp,
                    tc.tile_pool(name="cps", bufs=2, space="PSUM") as cpsp,
                    tc.tile_pool(name="s4ps", bufs=1, space="PSUM") as s4ps,
                ):
                    # zeros rows 0:64 + recip row 64; gpsimd partition all-reduce
                    # broadcasts the recip row to all partitions
                    zt = att.tile([65, 1024], f32, name="zt", bufs=1)
                    nc.vector.memset(zt[0:HD, :], 0.0)

                    def stage1_unit(b, half, tc_i):
                        """QKV projection for one 512-token chunk of batch b."""
                        yq, yk, vpb = ytq[b % 2], ytk[b % 2], vp[b % 2]
                        sh = 2 * b + half
                        rows = ag_out[sh * D:(sh + 1) * D, :]
                        tsl_l = slice(tc_i * 512, (tc_i + 1) * 512)
                        tsl_g = slice(half * SH_T + tc_i * 512,
                                      half * SH_T + (tc_i + 1) * 512)
                        xt_t = s1x.tile([P, DCH, 512], f16, name="xt_t")
                        nc.sync.dma_start(
                            xt_t[:],
                            rows[:, tsl_l].rearrange("(c p) t -> p c t", p=P))
                        for qk in range(2):  # Q then K features
                            ps = s1ps.tile([P, 512], f32, name="s1p")
                            for i in range(DCH):
                                nc.tensor.matmul(
                                    ps[:], wqk_t[:, i, qk * P:(qk + 1) * P],
                                    xt_t[:, i, :],
                                    start=(i == 0), stop=(i == DCH - 1))
                            ydst = (yq if qk == 0 else yk)
                            nc.vector.tensor_scalar(
                                out=ydst[:, tsl_g], in0=ps[:],
                                scalar1=bqk_t[:, qk:qk + 1],
                                scalar2=None, op0=mybir.AluOpType.add)
                        for sub in range(4):  # V for 128-token subtiles
                            tt = (2 * half + tc_i) * 4 + sub
                            ps = s1ps.tile([P, 512], f32, name="s1p")
                            for i in range(DCH):
                                nc.tensor.matmul(
                                    ps[:, 0:P], xt_t[:, i, sub * P:(sub + 1) * P],
                                    wv_t[:, i, :],
                                    start=(i == 0), stop=(i == DCH - 1))
                            vpt = vpb[:, tt, :].rearrange("p (k c) -> p k c", k=2)
                            vb4 = vb_t[:].rearrange("p (k c) -> p k c", k=2)
                            nc.vector.tensor_tensor(
                                out=vpt[:, :, 0:HD],
                                in0=ps[:, 0:P].rearrange("p (k c) -> p k c", k=2),
                                in1=vb4[:, :, 0:HD],
                                op=mybir.AluOpType.add)

                    def emit_norm(b, qa, cps_e, cps_o):
                        ctb = ct[b % 2]
                        nc.vector.reciprocal(zt[64:65, 0:512], cps_e[64:65, :])
                        nc.vector.reciprocal(zt[64:65, 512:1024], cps_o[64:65, :])
                        rbc = att.tile([65, 1024], f32, name="rbc", bufs=2)
                        nc.gpsimd.partition_all_reduce(
                            rbc[:], zt[:], channels=65,
                            reduce_op=bass_isa.ReduceOp.add)
                        nc.vector.tensor_mul(ctb[0:HD, qa], cps_e[0:HD, :],
                                             rbc[0:HD, 0:512])
                        cttmp = att.tile([HD, 512], f16, name="cttmp", bufs=1)
                        nc.vector.tensor_mul(cttmp[:], cps_o[0:HD, :],
                                             rbc[0:HD, 512:1024])
                        nc.sync.dma_start(ctb[HD:P, qa], cttmp[:])

                    # one projection token tile half -> rs_in rows
                    def proj_step(b, tt, half):
                        def f():
                            tsl = slice(tt * P, (tt + 1) * P)
                            ps = s4ps.tile([P, 512], f32, name="s4p")
                            nc.tensor.matmul(
                                ps[:], ct[b % 2][:, tsl],
                                wp_t[:, half * 512:(half + 1) * 512],
                                start=True, stop=True)
                            o_sb = s4o.tile([P, 512], f16, name="o_sb", bufs=4)
                            nc.vector.tensor_copy(o_sb[:], ps[:])
                            nc.sync.dma_start(
                                rs_in[b][tt * P:(tt + 1) * P,
                                         half * 512:(half + 1) * 512],
                                o_sb[:])
                        return f

                    def emit_pv(cps_e, cps_o, vpb, kc, ppt):
                        nc.tensor.matmul(cps_e[:], vpb[:, kc, 0:65],
                                         ppt[:, 0:512],
                                         start=(kc == 0), stop=(kc == NTT - 1))
                        nc.tensor.matmul(cps_o[:], vpb[:, kc, 65:130],
                                         ppt[:, 512:1024],
                                         start=(kc == 0), stop=(kc == NTT - 1))

                    def emit_rs(b):
                        nc.gpsimd.collective_compute(
                            "ReduceScatter", mybir.AluOpType.add,
                            replica_groups=RG8,
                            ins=[rs_in[b][:]], outs=[rs_out[b][:]])
                        ob = s4o.tile([P, 2, D], f16, name="ob", bufs=2)
                        nc.sync.dma_start(
                            ob[:],
                            rs_out[b][:].rearrange("(k p) d -> p k d", p=P))
                        nc.sync.dma_start(
                            out[b * 256:(b + 1) * 256, :].rearrange(
                                "(k p) d -> p k d", p=P), ob[:])

                    for u in range(4):
                        stage1_unit(0, u // 2, u % 2)
                    norm_pending = None
                    proj_queue = []   # (closure, rs_batch_or_None)
                    rs_ready = []
                    s1_queue = []
                    for b in range(B):
                        if b + 1 < B:
                            s1_queue = [(b + 1, u // 2, u % 2) for u in range(4)]
                        yq, yk, vpb = ytq[b % 2], ytk[b % 2], vp[b % 2]
                        for qc in range(4):  # 512-wide query chunks
                            qa = slice(qc * 512, (qc + 1) * 512)
                            cps_e = cps_o = None
                            pv_pending = None
                            for kc in range(NTT):
                                ksl = slice(kc * P, (kc + 1) * P)
                                spt = sptp.tile([P, 1024], f32, name="spt")
                                nc.tensor.matmul(spt[:, 0:512], yk[0:HD, ksl],
                                                 yq[0:HD, qa],
                                                 start=True, stop=True)
                                nc.tensor.matmul(spt[:, 512:1024], yk[HD:P, ksl],
                                                 yq[HD:P, qa],
                                                 start=True, stop=True)
                                ppt = att.tile([P, 1024], f16, name="ppt", bufs=4)
                                nc.scalar.activation(ppt[:], spt[:], AF.Exp,
                                                     scale=SCALE)
                                if kc == 1 and norm_pending is not None:
                                    emit_norm(*norm_pending)
                                    norm_pending = None
                                    if rs_ready:
                                        emit_rs(rs_ready.pop(0))
                                if pv_pending is not None:
                                    if cps_e is None:
                                        cps_e = cpsp.tile([65, 512], f32,
                                                          name="cps")
                                        cps_o = cpsp.tile([65, 512], f32,
                                                          name="cps")
                                    emit_pv(cps_e, cps_o, vpb, pv_pending[0],
                                            pv_pending[1])
                                pv_pending = (kc, ppt)
                                if proj_queue and kc >= 3:
                                    fn, rsb = proj_queue.pop(0)
                                    fn()
                                    if rsb is not None:
                                        rs_ready.append(rsb)
                            emit_pv(cps_e, cps_o, vpb, pv_pending[0],
                                    pv_pending[1])
                            norm_pending = (b, qa, cps_e, cps_o)
                            for tt in range(qc * 4, (qc + 1) * 4):
                                proj_queue.append((proj_step(b, tt, 0), None))
                                proj_queue.append((proj_step(b, tt, 1), None))
                            if s1_queue:  # drip next batch's QKV between sweeps
                                stage1_unit(*s1_queue.pop(0))
                        # tag batch b's last projection step so the RS fires
                        # once it has drained (at a later norm point)
                        fn0, _ = proj_queue[-1]
                        proj_queue[-1] = (fn0, b)
                    emit_norm(*norm_pending)
                    for fn, rsb in proj_queue:
                        fn()
                        if rsb is not None:
                            rs_ready.append(rsb)
                    for rsb in rs_ready:
                        emit_rs(rsb)

    nc.finalize()
    return nc


def get_nc(reps=1):
    global _CACHED_NC
    if reps != 1:
        return build_nc(reps)
    if _CACHED_NC is None:
        _CACHED_NC = build_nc()
    return _CACHED_NC


def make_in_maps(x, w_qkv, b_qkv, w_proj):
    """Host-side sharding: one input dict per core (all tensor I/O fp16)."""
    xf = np.asarray(x, np.float32).reshape(B * S, D)
    w3 = np.asarray(w_qkv, np.float32).reshape(D, 3, H_TOT, HD)
    b3 = np.asarray(b_qkv, np.float32).reshape(3, H_TOT, HD)
    wpr = np.asarray(w_proj, np.float32).reshape(H_TOT, HD, D)
    in_maps = []
    for c in range(8):
        hs = slice(2 * c, 2 * c + 2)
        xs_c = xf[c * SH_T:(c + 1) * SH_T].T
        wqk_c = np.concatenate(
            [w3[:, 0, hs].reshape(D, P), w3[:, 1, hs].reshape(D, P)], axis=1)
        wv_c = w3[:, 2, hs].reshape(D, P)
        wp_c = wpr[hs].reshape(P, D)
        bqk_c = np.stack([b3[0, hs].reshape(P), b3[1, hs].reshape(P)], axis=1)
        vb_c = np.zeros((P, 130), np.float32)
        vb_c[:, 0:HD] = b3[2, 2 * c]
        vb_c[:, 65:65 + HD] = b3[2, 2 * c + 1]
        blob = np.concatenate(
            [a.astype(np.float16).reshape(-1)
             for a in (xs_c, wqk_c, wv_c, wp_c, bqk_c, vb_c)]).reshape(1, -1)
        assert blob.shape[1] == NBLOB
        in_maps.append({"blob": blob})
    return in_maps


def assemble(results, b_proj):
    out = np.empty((B, S, D), np.float32)
    bp = np.asarray(b_proj, np.float32)
    for c in range(8):
        oc = np.asarray(results[c]["out"], np.float32)  # [B*256, D]
        for b in range(B):
            out[b, c * 256:(c + 1) * 256] = oc[b * 256:(b + 1) * 256]
    return out + bp


def kernel(x, w_qkv, b_qkv, w_proj, b_proj):
    nc = get_nc()
    in_maps = make_in_maps(x, w_qkv, b_qkv, w_proj)
    res = bass_utils.run_bass_kernel_spmd(nc, in_maps, core_ids=list(range(8)),
                                          trace=False)
    return assemble(res.results, b_proj)



# revision 7
# speedup vs baseline: 1.2888x; 1.2888x over previous
"""Multi-head self-attention (B=4, S=2048, D=1024, H=16) on 8 TRN2 NeuronCores.

Sharding: head-pair tensor parallel. Core c owns heads {2c, 2c+1} for ALL
batches; weights shard 8-way with zero duplication. The FULL input x is
shipped to every core (transposed [D, B*S], fp16) so no AllGather is needed
on-device (the previous AG of 16.8 MB cost ~265 us and stalled the PE for
the first ~290 us of every run); stage 1 streams x straight from DRAM one
512-token chunk at a time. Each core computes QKV, attention and its 2-head
partial projection for all 4 batches; a per-batch ReduceScatter sums the
partials and leaves each core with 256 final rows per batch (fp16). Host
adds b_proj.

Per-core dataflow per batch b (matmuls fp16 operands, fp32 PSUM):
  stage 1: yt_q/yt_k = [Q^T;K^T] [128f, 2048t], vp = V+[bias|ones] [2048t,130]
           (emitted in 4 chunks, dripped between attention query sweeps)
  stage 2: S^T[k,q] for the head pair (d=64 contraction), exp on ACT -> fp16
  stage 3: C~^T = [V_h|1]^T P^T (psum row 64 = softmax denom);
           recip -> gpsimd partition-broadcast -> DVE normalize -> ct fp16
  stage 4: out_partial = ct^T @ wp -> fp16 -> DRAM, dripped between S/exp
           steps; ReduceScatter(batch) once its last tile lands
"""
import numpy as np

import concourse.bacc as bacc
import concourse.tile as tile
from concourse import bass_isa, mybir
from concourse import bass_utils

P = 128
B, S, D = 4, 2048, 1024
H_TOT, HD = 16, 64
SCALE = HD ** -0.5
SH_T = 1024        # tokens per shard (B*S/8)
DCH = D // P       # 8 contraction chunks
NTT = S // P       # 16 token tiles per batch
f32 = mybir.dt.float32
f16 = mybir.dt.float16
AF = mybir.ActivationFunctionType
RG8 = [[0, 1, 2, 3, 4, 5, 6, 7]]

_CACHED_NC = None


# packed single-input blob layout (f16 element offsets)
BS = B * S          # 8192 tokens, all batches
OFF_XS = 0
OFF_WQK = OFF_XS + D * BS
OFF_WV = OFF_WQK + D * 2 * P
OFF_WP = OFF_WV + D * P
OFF_BQK = OFF_WP + P * D
OFF_VB = OFF_BQK + P * 2
NBLOB = OFF_VB + P * 130


def build_nc(reps=1):
    nc = bacc.Bacc(trn_type="TRN2", target_bir_lowering=False, debug=False,
                   num_devices=8, enable_partition_id=False)
    blob = nc.dram_tensor("blob", [1, NBLOB], f16, kind="ExternalInput").ap()
    xs = blob[0:1, OFF_XS:OFF_WQK]
    wqk = blob[0:1, OFF_WQK:OFF_WV]
    wv = blob[0:1, OFF_WV:OFF_WP]
    wp = blob[0:1, OFF_WP:OFF_BQK]
    bqk = blob[0:1, OFF_BQK:OFF_VB]
    vb = blob[0:1, OFF_VB:NBLOB]
    out = nc.dram_tensor("out", [B * 256, D], f16, kind="ExternalOutput").ap()

    # full x, transposed [D, BS]; row d = (c p): partition view [p, c, t]
    xs_v = xs.rearrange("o (c p t) -> p (o c) t", p=P, t=BS)
    rs_in = [nc.dram_tensor(f"rs_in{b}", [S, D], f16, kind="Internal").ap()
             for b in range(B)]
    rs_out = [nc.dram_tensor(f"rs_out{b}", [256, D], f16,
                             kind="Internal").ap() for b in range(B)]

    with tile.TileContext(nc) as tc:
        with tc.tile_pool(name="persist", bufs=1) as pp:
            # double-buffered per-batch persistent tensors (b%2)
            ytq = [pp.tile([P, S], f16, name=f"ytq{i}") for i in range(2)]
            ytk = [pp.tile([P, S], f16, name=f"ytk{i}") for i in range(2)]
            vp = [pp.tile([P, NTT, 130], f16, name=f"vp{i}") for i in range(2)]
            ct = [pp.tile([P, S], f16, name=f"ct{i}") for i in range(2)]
            wqk_t = pp.tile([P, DCH, 2 * P], f16, name="wqk_t")
            wv_t = pp.tile([P, DCH, P], f16, name="wv_t")
            wp_t = pp.tile([P, D], f16, name="wp_t")
            bqk16 = pp.tile([P, 2], f16, name="bqk16")
            vb16 = pp.tile([P, 130], f16, name="vb16")
            bqk_t = pp.tile([P, 2], f32, name="bqk_t")
            vb_t = pp.tile([P, 130], f32, name="vb_t")

            # benchmarking support: reps>1 re-executes the whole body
            # (AllGather + 4 batches + ReduceScatters) serially
            for _rep in range(reps):
                # stage 0: weights to SBUF; x shard -> internal dram -> AllGather
                nc.scalar.dma_start(wqk_t[:],
                                    wqk.rearrange("o (c p f) -> p (o c) f",
                                                  p=P, f=2 * P))
                nc.scalar.dma_start(wv_t[:],
                                    wv.rearrange("o (c p f) -> p (o c) f",
                                                 p=P, f=P))
                nc.scalar.dma_start(wp_t[:],
                                    wp.rearrange("o (p f) -> (o p) f", p=P))
                nc.scalar.dma_start(bqk16[:],
                                    bqk.rearrange("o (p a) -> (o p) a", p=P))
                nc.scalar.dma_start(vb16[:],
                                    vb.rearrange("o (p a) -> (o p) a", p=P))
                nc.vector.tensor_copy(bqk_t[:], bqk16[:])
                nc.vector.tensor_copy(vb_t[:], vb16[:])
                # the softmax-denominator ones columns of vp (written once)
                for i in range(2):
                    nc.vector.memset(vp[i][:, :, HD:HD + 1], 1.0)
                    nc.vector.memset(vp[i][:, :, 129:130], 1.0)

                with (
                    tc.tile_pool(name="s1x", bufs=2) as s1x,
                    tc.tile_pool(name="s1ps", bufs=1, space="PSUM") as s1ps,
                    tc.tile_pool(name="att", bufs=1) as att,
                    tc.tile_pool(name="s4o", bufs=2) as s4o,
                    tc.tile_pool(name="spt", bufs=2, space="PSUM") as sptp,
                    tc.tile_pool(name="cps", bufs=2, space="PSUM") as cpsp,
                    tc.tile_pool(name="s4ps", bufs=1, space="PSUM") as s4ps,
                ):
                    # zeros rows 0:64 + recip row 64; gpsimd partition all-reduce
                    # broadcasts the recip row to all partitions
                    zt = att.tile([65, 1024], f32, name="zt", bufs=1)
                    nc.vector.memset(zt[0:HD, :], 0.0)

                    def stage1_unit(b, half, tc_i):
                        """QKV projection for one 512-token chunk of batch b."""
                        yq, yk, vpb = ytq[b % 2], ytk[b % 2], vp[b % 2]
                        t0 = (2 * b + half) * SH_T + tc_i * 512
                        tsl_g = slice(half * SH_T + tc_i * 512,
                                      half * SH_T + (tc_i + 1) * 512)
                        xt_t = s1x.tile([P, DCH, 512], f16, name="xt_t")
                        nc.sync.dma_start(xt_t[:], xs_v[:, :, t0:t0 + 512])
                        for qk in range(2):  # Q then K features
                            ps = s1ps.tile([P, 512], f32, name="s1p")
                            for i in range(DCH):
                                nc.tensor.matmul(
                                    ps[:], wqk_t[:, i, qk * P:(qk + 1) * P],
                                    xt_t[:, i, :],
                                    start=(i == 0), stop=(i == DCH - 1))
                            ydst = (yq if qk == 0 else yk)
                            nc.vector.tensor_scalar(
                                out=ydst[:, tsl_g], in0=ps[:],
                                scalar1=bqk_t[:, qk:qk + 1],
                                scalar2=None, op0=mybir.AluOpType.add)
                        for sub in range(4):  # V for 128-token subtiles
                            tt = (2 * half + tc_i) * 4 + sub
                            ps = s1ps.tile([P, 512], f32, name="s1p")
                            for i in range(DCH):
                                nc.tensor.matmul(
                                    ps[:, 0:P], xt_t[:, i, sub * P:(sub + 1) * P],
                                    wv_t[:, i, :],
                                    start=(i == 0), stop=(i == DCH - 1))
                            vpt = vpb[:, tt, :].rearrange("p (k c) -> p k c", k=2)
                            vb4 = vb_t[:].rearrange("p (k c) -> p k c", k=2)
                            nc.vector.tensor_tensor(
                                out=vpt[:, :, 0:HD],
                                in0=ps[:, 0:P].rearrange("p (k c) -> p k c", k=2),
                                in1=vb4[:, :, 0:HD],
                                op=mybir.AluOpType.add)

                    def emit_norm(b, qa, cps_e, cps_o):
                        ctb = ct[b % 2]
                        nc.vector.reciprocal(zt[64:65, 0:512], cps_e[64:65, :])
                        nc.vector.reciprocal(zt[64:65, 512:1024], cps_o[64:65, :])
                        rbc = att.tile([65, 1024], f32, name="rbc", bufs=2)
                        nc.gpsimd.partition_all_reduce(
                            rbc[:], zt[:], channels=65,
                            reduce_op=bass_isa.ReduceOp.add)
                        nc.vector.tensor_mul(ctb[0:HD, qa], cps_e[0:HD, :],
                                             rbc[0:HD, 0:512])
                        cttmp = att.tile([HD, 512], f16, name="cttmp", bufs=1)
                        nc.vector.tensor_mul(cttmp[:], cps_o[0:HD, :],
                                             rbc[0:HD, 512:1024])
                        nc.sync.dma_start(ctb[HD:P, qa], cttmp[:])

                    # one projection token tile half -> rs_in rows
                    def proj_step(b, tt, half):
                        def f():
                            tsl = slice(tt * P, (tt + 1) * P)
                            ps = s4ps.tile([P, 512], f32, name="s4p")
                            nc.tensor.matmul(
                                ps[:], ct[b % 2][:, tsl],
                                wp_t[:, half * 512:(half + 1) * 512],
                                start=True, stop=True)
                            o_sb = s4o.tile([P, 512], f16, name="o_sb", bufs=4)
                            nc.vector.tensor_copy(o_sb[:], ps[:])
                            nc.sync.dma_start(
                                rs_in[b][tt * P:(tt + 1) * P,
                                         half * 512:(half + 1) * 512],
                                o_sb[:])
                        return f

                    def emit_pv(cps_e, cps_o, vpb, kc, ppt):
                        nc.tensor.matmul(cps_e[:], vpb[:, kc, 0:65],
                                         ppt[:, 0:512],
                                         start=(kc == 0), stop=(kc == NTT - 1))
                        nc.tensor.matmul(cps_o[:], vpb[:, kc, 65:130],
                                         ppt[:, 512:1024],
                                         start=(kc == 0), stop=(kc == NTT - 1))

                    def emit_rs(b):
                        nc.gpsimd.collective_compute(
                            "ReduceScatter", mybir.AluOpType.add,
                            replica_groups=RG8,
                            ins=[rs_in[b][:]], outs=[rs_out[b][:]])
                        ob = s4o.tile([P, 2, D], f16, name="ob", bufs=2)
                        nc.sync.dma_start(
                            ob[:],
                            rs_out[b][:].rearrange("(k p) d -> p k d", p=P))
                        nc.sync.dma_start(
                            out[b * 256:(b + 1) * 256, :].rearrange(
                                "(k p) d -> p k d", p=P), ob[:])

                    for u in range(4):
                        stage1_unit(0, u // 2, u % 2)
                    norm_pending = None
                    proj_queue = []   # (closure, rs_batch_or_None)
                    rs_ready = []
                    s1_queue = []
                    for b in range(B):
                        if b + 1 < B:
                            s1_queue = [(b + 1, u // 2, u % 2) for u in range(4)]
                        yq, yk, vpb = ytq[b % 2], ytk[b % 2], vp[b % 2]
                        for qc in range(4):  # 512-wide query chunks
                            qa = slice(qc * 512, (qc + 1) * 512)
                            cps_e = cps_o = None
                            pv_pending = None
                            for kc in range(NTT):
                                ksl = slice(kc * P, (kc + 1) * P)
                                spt = sptp.tile([P, 1024], f32, name="spt")
                                nc.tensor.matmul(spt[:, 0:512], yk[0:HD, ksl],
                                                 yq[0:HD, qa],
                                                 start=True, stop=True)
                                nc.tensor.matmul(spt[:, 512:1024], yk[HD:P, ksl],
                                                 yq[HD:P, qa],
                                                 start=True, stop=True)
                                ppt = att.tile([P, 1024], f16, name="ppt", bufs=4)
                                nc.scalar.activation(ppt[:], spt[:], AF.Exp,
                                                     scale=SCALE)
                                if kc == 1 and norm_pending is not None:
                                    emit_norm(*norm_pending)
                                    norm_pending = None
                                    if rs_ready:
                                        emit_rs(rs_ready.pop(0))
                                if pv_pending is not None:
                                    if cps_e is None:
                                        cps_e = cpsp.tile([65, 512], f32,
                                                          name="cps")
                                        cps_o = cpsp.tile([65, 512], f32,
                                                          name="cps")
                                    emit_pv(cps_e, cps_o, vpb, pv_pending[0],
                                            pv_pending[1])
                                pv_pending = (kc, ppt)
                                if proj_queue and kc >= 3:
                                    fn, rsb = proj_queue.pop(0)
                                    fn()
                                    if rsb is not None:
                                        rs_ready.append(rsb)
                            emit_pv(cps_e, cps_o, vpb, pv_pending[0],
                                    pv_pending[1])
                            norm_pending = (b, qa, cps_e, cps_o)
                            for tt in range(qc * 4, (qc + 1) * 4):
                                proj_queue.append((proj_step(b, tt, 0), None))
                                proj_queue.append((proj_step(b, tt, 1), None))
                            if s1_queue:  # drip next batch's QKV between sweeps
                                stage1_unit(*s1_queue.pop(0))
                        # tag batch b's last projection step so the RS fires
                        # once it has drained (at a later norm point)
                        fn0, _ = proj_queue[-1]
                        proj_queue[-1] = (fn0, b)
                    emit_norm(*norm_pending)
                    for fn, rsb in proj_queue:
                        fn()
                        if rsb is not None:
                            rs_ready.append(rsb)
                    for rsb in rs_ready:
                        emit_rs(rsb)

    nc.finalize()
    return nc


def get_nc(reps=1):
    global _CACHED_NC
    if reps != 1:
        return build_nc(reps)
    if _CACHED_NC is None:
        _CACHED_NC = build_nc()
    return _CACHED_NC


def make_in_maps(x, w_qkv, b_qkv, w_proj):
    """Host-side sharding: one input dict per core (all tensor I/O fp16)."""
    xf = np.asarray(x, np.float32).reshape(B * S, D)
    w3 = np.asarray(w_qkv, np.float32).reshape(D, 3, H_TOT, HD)
    b3 = np.asarray(b_qkv, np.float32).reshape(3, H_TOT, HD)
    wpr = np.asarray(w_proj, np.float32).reshape(H_TOT, HD, D)
    xs_c = np.ascontiguousarray(xf.T).astype(np.float16)  # [D, BS], all cores
    in_maps = []
    for c in range(8):
        hs = slice(2 * c, 2 * c + 2)
        wqk_c = np.concatenate(
            [w3[:, 0, hs].reshape(D, P), w3[:, 1, hs].reshape(D, P)], axis=1)
        wv_c = w3[:, 2, hs].reshape(D, P)
        wp_c = wpr[hs].reshape(P, D)
        bqk_c = np.stack([b3[0, hs].reshape(P), b3[1, hs].reshape(P)], axis=1)
        vb_c = np.zeros((P, 130), np.float32)
        vb_c[:, 0:HD] = b3[2, 2 * c]
        vb_c[:, 65:65 + HD] = b3[2, 2 * c + 1]
        blob = np.concatenate(
            [a.astype(np.float16).reshape(-1)
             for a in (xs_c, wqk_c, wv_c, wp_c, bqk_c, vb_c)]).reshape(1, -1)
        assert blob.shape[1] == NBLOB
        in_maps.append({"blob": blob})
    return in_maps


def assemble(results, b_proj):
    out = np.empty((B, S, D), np.float32)
    bp = np.asarray(b_proj, np.float32)
    for c in range(8):
        oc = np.asarray(results[c]["out"], np.float32)  # [B*256, D]
        for b in range(B):
            out[b, c * 256:(c + 1) * 256] = oc[b * 256:(b + 1) * 256]
    return out + bp


def kernel(x, w_qkv, b_qkv, w_proj, b_proj):
    nc = get_nc()
    in_maps = make_in_maps(x, w_qkv, b_qkv, w_proj)
    res = bass_utils.run_bass_kernel_spmd(nc, in_maps, core_ids=list(range(8)),
                                          trace=False)
    return assemble(res.results, b_proj)

